# revision 1
# baseline (speedup 1.0000x reference)
"""Bass/Trainium2 kernel for BasicMOE (soft-router MoE with broadcast-bug collapse).

The reference computes
    w = softmax(x @ Wg + bg)                    [B, E]
    y = einsum('bi,eio->beo', x, We) + be       [B, E, O]
    total = einsum('be,beo->o', w, y)           [O]
    out = broadcast(total, [B, O])
which algebraically collapses to
    z = w.T @ x                                 [E, IN]
    s = w.sum(axis=0)                           [E]
    total = einsum('ei,eio->o', z, We) + s @ be [O]
so the kernel never materializes [B, E, O].  The cost is a single streaming
pass over We (1 GiB fp32), expert-sharded across 8 cores (128 MiB/core).

Sharding: We/be sharded on the expert axis (2 experts/core, shipped as a
bf16 hi/lo pair so the PE streams at 1 cycle/row with fp32-quality math).
The gating is batch-sharded: each core computes softmax/z-partials for its
own 128 rows of x, then an on-device ReduceScatter over the expert axis
hands core c the fully-reduced z rows [2c, 2c+2) (s rides along in a spare
column).  Each core emits its partial total [O]; the host sums the 8
partials and broadcasts to [B, O].
"""

import ml_dtypes
import numpy as np

import concourse.bass as bass
import concourse.mybir as mybir
import concourse.tile as tile
from concourse.bass_utils import run_bass_kernel_spmd
from concourse.masks import make_identity

BF16 = np.dtype(ml_dtypes.bfloat16)

B, IN, OUT, E = 1024, 4096, 4096, 16
NCORES = 8
EPC = E // NCORES          # experts per core = 2
KPC = EPC * IN             # contraction rows per core = 8192
NKT = KPC // 128           # we k-tiles per core = 64
NIT = IN // 128            # i-tiles = 32
NBT = B // 128             # b-tiles = 8
NOC = OUT // 512           # output chunks = 8
FP = mybir.dt.float32

# DMA batching for the We stream: K-tiles per slab DMA.
SLAB_KT = 2                # 2 MiB (bf16) per dma_start
SLAB_BUFS = 4
X_BUFS = 2
XT_BUFS = 3


CCW = IN + 128             # ReduceScatter payload width: z | s | pad


def _moe_device_program(nc, xs_d, xts_d, wg_d, bg_d, wehi_d, welo_d, be_d, out_d):
    with tile.TileContext(nc) as tc:
        with (
            tc.tile_pool(name="singles", bufs=1) as singles,
            tc.tile_pool(name="small", bufs=4) as small,
            tc.tile_pool(name="slab_pool", bufs=SLAB_BUFS + 1) as slab_pool,
            tc.tile_pool(name="slab_lo_pool", bufs=SLAB_BUFS) as slab_lo_pool,
            tc.tile_pool(name="be_pool", bufs=2) as be_pool,
            tc.tile_pool(name="out_pool", bufs=2) as out_pool,
            tc.tile_pool(name="ccdram", bufs=1, space="DRAM") as dram_pool,
        ):
            # ---- constants / small resident tensors ----
            wg_sbuf = singles.tile([128, NIT, E], FP)          # [128, 32, 16]
            nc.gpsimd.dma_start(
                out=wg_sbuf, in_=wg_d.rearrange("p (t e) -> p t e", t=NIT)
            )
            bg_sbuf = singles.tile([E, 1], FP)
            nc.gpsimd.dma_start(out=bg_sbuf, in_=bg_d)
            ident = singles.tile([E, E], FP)
            make_identity(nc, ident)
            ones128 = singles.tile([128, 1], FP)
            nc.vector.memset(ones128, 1.0)

            logitsT_s = singles.tile([E, 128], FP)             # [16, 128]
            zcat = singles.tile([E, CCW], FP)                  # z | s | pad
            # After the zin store, zcat is dead; the reduced rows land in
            # its first EPC partitions (Tile serializes the WAR hazard).
            zr_sbuf = zcat[0:EPC, :]
            zT_hi = singles.tile([128, NIT, EPC], mybir.dt.bfloat16)
            zT_lo = singles.tile([128, NIT, EPC], mybir.dt.bfloat16)
            xs_tile = singles.tile([128, IN], FP)              # batch shard
            xts_sbuf = singles.tile([128, NIT, 128], FP)       # its transpose
            nc.gpsimd.dma_start(out=xs_tile, in_=xs_d)
            nc.gpsimd.dma_start(
                out=xts_sbuf, in_=xts_d.rearrange("(t p) b -> p t b", p=128)
            )
            zin = dram_pool.tile([E, CCW], FP)
            zred = dram_pool.tile([EPC, CCW], FP)

            with (
                tc.tile_pool(name="psA", bufs=1, space="PSUM") as psA_pool,
                tc.tile_pool(name="ps_tr", bufs=2, space="PSUM") as tr_pool,
                tc.tile_pool(name="ps_s", bufs=1, space="PSUM") as s_pool,
                tc.tile_pool(name="ps_z", bufs=2, space="PSUM") as z_pool,
                tc.tile_pool(name="ps_warm", bufs=1, space="PSUM") as warm_pool,
            ):
                # Scratch PSUM target for "wait absorber" matmuls.  The PE's
                # LDWEIGHTS slot encodes a single semaphore wait, so any
                # matmul whose operands complete on two different semaphores
                # fails walrus codegen.  Each absorber below reads exactly one
                # not-yet-observed producer so the real matmuls that follow
                # need at most one wait.
                warm = warm_pool.tile([E, E], FP)

                def absorb(src_col):
                    return nc.tensor.matmul(
                        warm[0:1, 0:1], src_col, src_col,
                        start=True, stop=True, skip_group_check=True,
                    )

                absorb(wg_sbuf[:, 0, 0:1])          # wg DMA
                nc.tensor.transpose(warm, ident, ident)  # ident (gpsimd)
                absorb(xs_tile[:, 0:1])             # xs DMA
                absorb(xts_sbuf[:, 0, 0:1])         # xts DMA

                # ---- Phase A: local logitsT = Wg.T @ xs.T  [16, 128]
                lgA = psA_pool.tile([E, 128], FP)
                for it in range(NIT):
                    nc.tensor.matmul(
                        lgA, wg_sbuf[:, it, :], xts_sbuf[:, it, :],
                        start=(it == 0), stop=(it == NIT - 1),
                    )
                nc.vector.tensor_scalar_add(logitsT_s, lgA, bg_sbuf)

                # ---- Phase B: softmax for the local 128 rows, z/s partials
                lg_ps = tr_pool.tile([128, E], FP, tag="tr")
                nc.tensor.transpose(lg_ps, logitsT_s, ident)
                mx = small.tile([128, 1], FP)
                nc.vector.reduce_max(mx, lg_ps, axis=mybir.AxisListType.X)
                negmx = small.tile([128, 1], FP)
                nc.vector.tensor_scalar_mul(negmx, mx, -1.0)
                wexp = small.tile([128, E], FP)
                ssum = small.tile([128, 1], FP)
                nc.scalar.activation(
                    wexp, lg_ps, mybir.ActivationFunctionType.Exp,
                    bias=negmx, accum_out=ssum,
                )
                rc = small.tile([128, 1], FP)
                nc.vector.reciprocal(rc, ssum)
                w_tile = small.tile([128, E], FP)
                nc.vector.tensor_scalar_mul(w_tile, wexp, rc)

                nc.vector.memset(zcat[:, IN:], 0.0)
                s_ps = s_pool.tile([E, 1], FP)
                nc.tensor.matmul(s_ps, w_tile, ones128)
                for c in range(IN // 512):
                    z_ps = z_pool.tile([E, 512], FP, tag="zc")
                    nc.tensor.matmul(
                        z_ps, w_tile, xs_tile[:, c * 512:(c + 1) * 512]
                    )
                    nc.vector.tensor_copy(zcat[:, c * 512:(c + 1) * 512], z_ps)
                    # Stream each chunk to the collective payload as soon as
                    # it lands so the store overlaps the remaining z matmuls.
                    nc.gpsimd.dma_start(
                        out=zin[:, c * 512:(c + 1) * 512],
                        in_=zcat[:, c * 512:(c + 1) * 512],
                    )
                nc.vector.tensor_copy(zcat[:, IN:IN + 1], s_ps)

                # ---- ReduceScatter over the expert axis: core c receives
                # rows [2c, 2c+2) of sum_cores(zcat) = its experts' z and s.
                nc.gpsimd.dma_start(out=zin[:, IN:], in_=zcat[:, IN:])
                nc.gpsimd.collective_compute(
                    "ReduceScatter", mybir.AluOpType.add,
                    replica_groups=[list(range(NCORES))],
                    ins=[zin], outs=[zred],
                )
                nc.gpsimd.dma_start(out=zr_sbuf, in_=zred)

                # ---- Phase C: zT tiles via PE transpose, split into bf16
                # hi/lo pair (z = hi + lo to ~2^-16 relative).
                absorb(zr_sbuf[:, 0:1])             # zr DMA
                for it in range(NIT):
                    zt_ps = tr_pool.tile([128, EPC], FP, tag="tr")
                    nc.tensor.transpose(
                        zt_ps, zr_sbuf[:, it * 128:(it + 1) * 128],
                        ident[0:EPC, 0:EPC],
                    )
                    nc.vector.tensor_copy(zT_hi[:, it, :], zt_ps)
                    nc.vector.tensor_sub(
                        zT_lo[:, it, :], zt_ps, zT_hi[:, it, :]
                    )
                a_s = absorb(zr_sbuf[0:1, IN:IN + 1])  # (s now lives in zr)
                a_zt = absorb(zT_lo[:, NIT - 1, 0:1])  # DVE last zT write

            # ---- Phase D: total[o] = sum_k z[k] We[k, o]  (+ s @ be)
            with tc.tile_pool(name="ps_tot", bufs=NOC, space="PSUM") as tot_pool:
                tots = [
                    tot_pool.tile([1, 512], FP, name=f"tot{ot}", tag="tot")
                    for ot in range(NOC)
                ]

                from concourse.tile_rust import add_dep_helper
                for ot in range(NOC):
                    be_t = be_pool.tile([EPC, 512], FP, name=f"bet{ot}",
                                        tag="bet")
                    nc.sync.dma_start(
                        out=be_t, in_=be_d[:, ot * 512:(ot + 1) * 512]
                    )
                    bm = nc.tensor.matmul(
                        tots[ot], zr_sbuf[0:EPC, IN:IN + 1], be_t,
                        start=True, stop=False,
                    )
                    if ot == 0:
                        # Keep the wait-absorbers ahead of the first phase-D
                        # matmul so it needs only the PSUM bank-release wait.
                        add_dep_helper(bm.ins, a_s.ins, False)
                        add_dep_helper(bm.ins, a_zt.ins, False)
                BF = mybir.dt.bfloat16
                wehi_r = wehi_d.rearrange("(n a p) o -> n p a o", a=SLAB_KT, p=128)
                welo_r = welo_d.rearrange("(n a p) o -> n p a o", a=SLAB_KT, p=128)
                for n in range(NKT // SLAB_KT):
                    slab_hi = slab_pool.tile([128, SLAB_KT, OUT], BF)
                    nc.sync.dma_start(out=slab_hi, in_=wehi_r[n])
                    slab_lo = slab_lo_pool.tile([128, SLAB_KT, OUT], BF)
                    nc.sync.dma_start(out=slab_lo, in_=welo_r[n])
                    for a in range(SLAB_KT):
                        k = n * SLAB_KT + a
                        e, it = divmod(k, NIT)
                        zh = zT_hi[:, it, e:e + 1]
                        zl = zT_lo[:, it, e:e + 1]
                        last = k == NKT - 1
                        for ot in range(NOC):
                            hi_chunk = slab_hi[:, a, ot * 512:(ot + 1) * 512]
                            lo_chunk = slab_lo[:, a, ot * 512:(ot + 1) * 512]
                            nc.tensor.matmul(
                                tots[ot], zh, hi_chunk, start=False, stop=False,
                            )
                            nc.tensor.matmul(
                                tots[ot], zl, hi_chunk, start=False, stop=False,
                            )
                            nc.tensor.matmul(
                                tots[ot], zh, lo_chunk,
                                start=False, stop=last,
                            )
                for ot in range(NOC):
                    oc = out_pool.tile([1, 512], FP, name=f"oc{ot}", tag="oc")
                    nc.vector.tensor_copy(oc, tots[ot])
                    nc.sync.dma_start(
                        out=out_d[0:1, ot * 512:(ot + 1) * 512], in_=oc
                    )


def _split_multi_waits(nc, keep=1):
    """Walrus encodes at most one semaphore wait per TPB instruction struct
    (S3_LW for matmul, PSEUDO_DMA_DIRECT2D for DMA, ...).  Tile's scheduler
    sometimes attaches 2-3 waits to one instruction; hoist the extras onto
    standalone same-engine EventSemaphore waits placed just before it --
    semantically identical (the engine sequencer blocks on them in order).
    """
    n = 0
    for f in nc.m.functions:
        for blk in f.blocks:
            new_insts = []
            for inst in blk.instructions:
                si = getattr(inst, "sync_info", None)
                waits = list(si.on_wait) if si and si.on_wait else []
                if len(waits) > keep:
                    for w in waits[:-keep]:
                        ev = mybir.InstEventSemaphore(
                            name=f"presplit_{n}_{inst.name}", ins=[], outs=[]
                        )
                        n += 1
                        ev.engine = inst.engine
                        ev.sync_info = mybir.SyncInfo(on_wait=[w], on_update=[])
                        ev.bass_nofuse = True
                        new_insts.append(ev)
                    si.on_wait = waits[-keep:]
                new_insts.append(inst)
            blk.instructions = new_insts
    return n


def build_bass(split_waits=True):
    nc = bass.Bass("TRN2", target_bir_lowering=False, num_devices=NCORES)
    xs_d = nc.dram_tensor("xs", [128, IN], FP, kind="ExternalInput").ap()
    xts_d = nc.dram_tensor("xts", [IN, 128], FP, kind="ExternalInput").ap()
    wg_d = nc.dram_tensor("wg", [128, NIT * E], FP, kind="ExternalInput").ap()
    bg_d = nc.dram_tensor("bg", [E, 1], FP, kind="ExternalInput").ap()
    BF = mybir.dt.bfloat16
    wehi_d = nc.dram_tensor("wehi", [KPC, OUT], BF, kind="ExternalInput").ap()
    welo_d = nc.dram_tensor("welo", [KPC, OUT], BF, kind="ExternalInput").ap()
    be_d = nc.dram_tensor("be", [EPC, OUT], FP, kind="ExternalInput").ap()
    out_d = nc.dram_tensor("out", [1, OUT], FP, kind="ExternalOutput").ap()
    _moe_device_program(nc, xs_d, xts_d, wg_d, bg_d, wehi_d, welo_d, be_d, out_d)
    if split_waits:
        _split_multi_waits(nc)
    return nc


def make_in_maps(x, Wg, bg, We, be):
    x = np.ascontiguousarray(np.asarray(x, dtype=np.float32))
    Wg = np.asarray(Wg, dtype=np.float32)
    bg = np.asarray(bg, dtype=np.float32)
    We = np.asarray(We, dtype=np.float32)
    be = np.asarray(be, dtype=np.float32)
    # Pre-rearranged gating weights: partition-major [128, NIT*E] so the
    # device load is a single contiguous-per-partition DMA.
    wg_c = np.ascontiguousarray(
        Wg.reshape(NIT, 128, E).transpose(1, 0, 2).reshape(128, NIT * E))
    bg_c = np.ascontiguousarray(bg).reshape(E, 1)
    in_maps = []
    for c in range(NCORES):
        loc = list(range(EPC * c, EPC * (c + 1)))
        we_c = We[loc[0]:loc[-1] + 1].reshape(KPC, OUT)
        we_hi = we_c.astype(BF16)
        we_lo = (we_c - we_hi.astype(np.float32)).astype(BF16)
        xs = np.ascontiguousarray(x[c * 128:(c + 1) * 128])
        in_maps.append({
            "xs": xs,
            "xts": np.ascontiguousarray(xs.T),
            "wg": wg_c,
            "bg": bg_c,
            "wehi": we_hi,
            "welo": we_lo,
            "be": np.ascontiguousarray(be[loc[0]:loc[-1] + 1]),
        })
    return in_maps


_NC_CACHE = None


def _get_nc():
    global _NC_CACHE
    if _NC_CACHE is None:
        _NC_CACHE = build_bass()
    return _NC_CACHE


def kernel(x, Wg, bg, We, be, **_ignored):
    in_maps = make_in_maps(x, Wg, bg, We, be)
    nc = _get_nc()
    res = run_bass_kernel_spmd(nc, in_maps, core_ids=list(range(NCORES)))
    total = np.zeros(OUT, dtype=np.float32)
    for r in res.results:
        total = total + r["out"].reshape(OUT).astype(np.float32)
    return np.ascontiguousarray(
        np.broadcast_to(total, (B, OUT)).astype(np.float32)
    )



# revision 2
# speedup vs baseline: 33.0681x; 33.0681x over previous
"""Bass/Trainium2 kernel for BasicMOE (soft-router MoE with broadcast-bug collapse).

The reference computes
    w = softmax(x @ Wg + bg)                    [B, E]
    y = einsum('bi,eio->beo', x, We) + be       [B, E, O]
    total = einsum('be,beo->o', w, y)           [O]
    out = broadcast(total, [B, O])
which algebraically collapses to
    z = w.T @ x                                 [E, IN]
    s = w.sum(axis=0)                           [E]
    total = einsum('ei,eio->o', z, We) + s @ be [O]
so the kernel never materializes [B, E, O].  The cost is a single streaming
pass over We, expert-sharded across 8 cores and shipped in bf16 (64 MiB/core)
— quantization error ~1e-3 relative, well inside the 2e-2 gate.

Sharding: We/be sharded on the expert axis (2 experts/core).  The gating is
batch-sharded: each core computes softmax/z-partials for its own 128 rows of
x, then an on-device ReduceScatter over the expert axis hands core c the
fully-reduced z rows [2c, 2c+2) (s rides along in a spare column).  Each core
emits its partial total [O]; the host sums the 8 partials and broadcasts to
[B, O].
"""

import ml_dtypes
import numpy as np

import concourse.bass as bass
import concourse.mybir as mybir
import concourse.tile as tile
from concourse.bass_utils import run_bass_kernel_spmd
from concourse.masks import make_identity

BF16 = np.dtype(ml_dtypes.bfloat16)

B, IN, OUT, E = 1024, 4096, 4096, 16
NCORES = 8
EPC = E // NCORES          # experts per core = 2
KPC = EPC * IN             # contraction rows per core = 8192
NKT = KPC // 128           # we k-tiles per core = 64
NIT = IN // 128            # i-tiles = 32
NBT = B // 128             # b-tiles = 8
NOC = OUT // 512           # output chunks = 8
FP = mybir.dt.float32

# DMA batching for the We stream: K-tiles per slab DMA (2 MiB bf16 each).
SLAB_KT = 2
SLAB_BUFS = 8              # 16 MiB of prefetch depth — covers the gating
                           # + ReduceScatter pre-phase without stalling DMA


CCW = IN + 128             # ReduceScatter payload width: z | s | pad


def _moe_device_program(nc, xs_d, xts_d, wg_d, bg_d, we_d, be_d, out_d):
    with tile.TileContext(nc) as tc:
        with (
            tc.tile_pool(name="singles", bufs=1) as singles,
            tc.tile_pool(name="small", bufs=4) as small,
            tc.tile_pool(name="slab_pool", bufs=SLAB_BUFS) as slab_pool,
            tc.tile_pool(name="be_pool", bufs=2) as be_pool,
            tc.tile_pool(name="out_pool", bufs=2) as out_pool,
            tc.tile_pool(name="ccdram", bufs=1, space="DRAM") as dram_pool,
        ):
            # ---- constants / small resident tensors ----
            wg_sbuf = singles.tile([128, NIT, E], FP)          # [128, 32, 16]
            nc.gpsimd.dma_start(
                out=wg_sbuf, in_=wg_d.rearrange("p (t e) -> p t e", t=NIT)
            )
            bg_sbuf = singles.tile([E, 1], FP)
            nc.gpsimd.dma_start(out=bg_sbuf, in_=bg_d)
            ident = singles.tile([E, E], FP)
            make_identity(nc, ident)
            ones128 = singles.tile([128, 1], FP)
            nc.vector.memset(ones128, 1.0)

            logitsT_s = singles.tile([E, 128], FP)             # [16, 128]
            zcat = singles.tile([E, CCW], FP)                  # z | s | pad
            # After the zin store, zcat is dead; the reduced rows land in
            # its first EPC partitions (Tile serializes the WAR hazard).
            zr_sbuf = zcat[0:EPC, :]
            zT = singles.tile([128, NIT, EPC], mybir.dt.bfloat16)
            xs_tile = singles.tile([128, IN], FP)              # batch shard
            xts_sbuf = singles.tile([128, NIT, 128], FP)       # its transpose
            nc.gpsimd.dma_start(out=xs_tile, in_=xs_d)
            nc.gpsimd.dma_start(
                out=xts_sbuf, in_=xts_d.rearrange("(t p) b -> p t b", p=128)
            )
            zin = dram_pool.tile([E, CCW], FP)
            zred = dram_pool.tile([EPC, CCW], FP)

            with (
                tc.tile_pool(name="psA", bufs=1, space="PSUM") as psA_pool,
                tc.tile_pool(name="ps_tr", bufs=2, space="PSUM") as tr_pool,
                tc.tile_pool(name="ps_s", bufs=1, space="PSUM") as s_pool,
                tc.tile_pool(name="ps_z", bufs=2, space="PSUM") as z_pool,
                tc.tile_pool(name="ps_warm", bufs=1, space="PSUM") as warm_pool,
            ):
                # Scratch PSUM target for "wait absorber" matmuls.  The PE's
                # LDWEIGHTS slot encodes a single semaphore wait, so any
                # matmul whose operands complete on two different semaphores
                # fails walrus codegen.  Each absorber below reads exactly one
                # not-yet-observed producer so the real matmuls that follow
                # need at most one wait.
                warm = warm_pool.tile([E, E], FP)

                def absorb(src_col):
                    return nc.tensor.matmul(
                        warm[0:1, 0:1], src_col, src_col,
                        start=True, stop=True, skip_group_check=True,
                    )

                absorb(wg_sbuf[:, 0, 0:1])          # wg DMA
                nc.tensor.transpose(warm, ident, ident)  # ident (gpsimd)
                absorb(xs_tile[:, 0:1])             # xs DMA
                absorb(xts_sbuf[:, 0, 0:1])         # xts DMA

                # ---- Phase A: local logitsT = Wg.T @ xs.T  [16, 128]
                lgA = psA_pool.tile([E, 128], FP)
                for it in range(NIT):
                    nc.tensor.matmul(
                        lgA, wg_sbuf[:, it, :], xts_sbuf[:, it, :],
                        start=(it == 0), stop=(it == NIT - 1),
                    )
                nc.vector.tensor_scalar_add(logitsT_s, lgA, bg_sbuf)

                # ---- Phase B: softmax for the local 128 rows, z/s partials
                lg_ps = tr_pool.tile([128, E], FP, tag="tr")
                nc.tensor.transpose(lg_ps, logitsT_s, ident)
                mx = small.tile([128, 1], FP)
                nc.vector.reduce_max(mx, lg_ps, axis=mybir.AxisListType.X)
                negmx = small.tile([128, 1], FP)
                nc.vector.tensor_scalar_mul(negmx, mx, -1.0)
                wexp = small.tile([128, E], FP)
                ssum = small.tile([128, 1], FP)
                nc.scalar.activation(
                    wexp, lg_ps, mybir.ActivationFunctionType.Exp,
                    bias=negmx, accum_out=ssum,
                )
                rc = small.tile([128, 1], FP)
                nc.vector.reciprocal(rc, ssum)
                w_tile = small.tile([128, E], FP)
                nc.vector.tensor_scalar_mul(w_tile, wexp, rc)

                nc.vector.memset(zcat[:, IN:], 0.0)
                s_ps = s_pool.tile([E, 1], FP)
                nc.tensor.matmul(s_ps, w_tile, ones128)
                for c in range(IN // 512):
                    z_ps = z_pool.tile([E, 512], FP, tag="zc")
                    nc.tensor.matmul(
                        z_ps, w_tile, xs_tile[:, c * 512:(c + 1) * 512]
                    )
                    nc.vector.tensor_copy(zcat[:, c * 512:(c + 1) * 512], z_ps)
                    # Stream each chunk to the collective payload as soon as
                    # it lands so the store overlaps the remaining z matmuls.
                    nc.gpsimd.dma_start(
                        out=zin[:, c * 512:(c + 1) * 512],
                        in_=zcat[:, c * 512:(c + 1) * 512],
                    )
                nc.vector.tensor_copy(zcat[:, IN:IN + 1], s_ps)

                # ---- ReduceScatter over the expert axis: core c receives
                # rows [2c, 2c+2) of sum_cores(zcat) = its experts' z and s.
                nc.gpsimd.dma_start(out=zin[:, IN:], in_=zcat[:, IN:])
                nc.gpsimd.collective_compute(
                    "ReduceScatter", mybir.AluOpType.add,
                    replica_groups=[list(range(NCORES))],
                    ins=[zin], outs=[zred],
                )
                nc.gpsimd.dma_start(out=zr_sbuf, in_=zred)

                # ---- Phase C: zT tiles via PE transpose, cast to bf16.
                absorb(zr_sbuf[:, 0:1])             # zr DMA
                for it in range(NIT):
                    zt_ps = tr_pool.tile([128, EPC], FP, tag="tr")
                    nc.tensor.transpose(
                        zt_ps, zr_sbuf[:, it * 128:(it + 1) * 128],
                        ident[0:EPC, 0:EPC],
                    )
                    nc.vector.tensor_copy(zT[:, it, :], zt_ps)
                a_s = absorb(zr_sbuf[0:1, IN:IN + 1])  # (s now lives in zr)
                a_zt = absorb(zT[:, NIT - 1, 0:1])     # DVE last zT write

            # ---- Phase D: total[o] = sum_k z[k] We[k, o]  (+ s @ be)
            with tc.tile_pool(name="ps_tot", bufs=NOC, space="PSUM") as tot_pool:
                tots = [
                    tot_pool.tile([1, 512], FP, name=f"tot{ot}", tag="tot")
                    for ot in range(NOC)
                ]

                from concourse.tile_rust import add_dep_helper
                for ot in range(NOC):
                    be_t = be_pool.tile([EPC, 512], FP, name=f"bet{ot}",
                                        tag="bet")
                    nc.sync.dma_start(
                        out=be_t, in_=be_d[:, ot * 512:(ot + 1) * 512]
                    )
                    bm = nc.tensor.matmul(
                        tots[ot], zr_sbuf[0:EPC, IN:IN + 1], be_t,
                        start=True, stop=False,
                    )
                    if ot == 0:
                        # Keep the wait-absorbers ahead of the first phase-D
                        # matmul so it needs only the PSUM bank-release wait.
                        add_dep_helper(bm.ins, a_s.ins, False)
                        add_dep_helper(bm.ins, a_zt.ins, False)
                BF = mybir.dt.bfloat16
                we_r = we_d.rearrange("(n a p) o -> n p a o", a=SLAB_KT, p=128)
                for n in range(NKT // SLAB_KT):
                    slab = slab_pool.tile([128, SLAB_KT, OUT], BF)
                    nc.sync.dma_start(out=slab, in_=we_r[n])
                    for a in range(SLAB_KT):
                        k = n * SLAB_KT + a
                        e, it = divmod(k, NIT)
                        zk = zT[:, it, e:e + 1]
                        last = k == NKT - 1
                        for ot in range(NOC):
                            nc.tensor.matmul(
                                tots[ot], zk,
                                slab[:, a, ot * 512:(ot + 1) * 512],
                                start=False, stop=last,
                            )
                for ot in range(NOC):
                    oc = out_pool.tile([1, 512], FP, name=f"oc{ot}", tag="oc")
                    nc.vector.tensor_copy(oc, tots[ot])
                    nc.sync.dma_start(
                        out=out_d[0:1, ot * 512:(ot + 1) * 512], in_=oc
                    )


def _split_multi_waits(nc, keep=1):
    """Walrus encodes at most one semaphore wait per TPB instruction struct
    (S3_LW for matmul, PSEUDO_DMA_DIRECT2D for DMA, ...).  Tile's scheduler
    sometimes attaches 2-3 waits to one instruction; hoist the extras onto
    standalone same-engine EventSemaphore waits placed just before it --
    semantically identical (the engine sequencer blocks on them in order).
    """
    n = 0
    for f in nc.m.functions:
        for blk in f.blocks:
            new_insts = []
            for inst in blk.instructions:
                si = getattr(inst, "sync_info", None)
                waits = list(si.on_wait) if si and si.on_wait else []
                if len(waits) > keep:
                    for w in waits[:-keep]:
                        ev = mybir.InstEventSemaphore(
                            name=f"presplit_{n}_{inst.name}", ins=[], outs=[]
                        )
                        n += 1
                        ev.engine = inst.engine
                        ev.sync_info = mybir.SyncInfo(on_wait=[w], on_update=[])
                        ev.bass_nofuse = True
                        new_insts.append(ev)
                    si.on_wait = waits[-keep:]
                new_insts.append(inst)
            blk.instructions = new_insts
    return n


def build_bass(split_waits=True):
    nc = bass.Bass("TRN2", target_bir_lowering=False, num_devices=NCORES)
    xs_d = nc.dram_tensor("xs", [128, IN], FP, kind="ExternalInput").ap()
    xts_d = nc.dram_tensor("xts", [IN, 128], FP, kind="ExternalInput").ap()
    wg_d = nc.dram_tensor("wg", [128, NIT * E], FP, kind="ExternalInput").ap()
    bg_d = nc.dram_tensor("bg", [E, 1], FP, kind="ExternalInput").ap()
    BF = mybir.dt.bfloat16
    we_d = nc.dram_tensor("we", [KPC, OUT], BF, kind="ExternalInput").ap()
    be_d = nc.dram_tensor("be", [EPC, OUT], FP, kind="ExternalInput").ap()
    out_d = nc.dram_tensor("out", [1, OUT], FP, kind="ExternalOutput").ap()
    _moe_device_program(nc, xs_d, xts_d, wg_d, bg_d, we_d, be_d, out_d)
    if split_waits:
        _split_multi_waits(nc)
    return nc


def make_in_maps(x, Wg, bg, We, be):
    x = np.ascontiguousarray(np.asarray(x, dtype=np.float32))
    Wg = np.asarray(Wg, dtype=np.float32)
    bg = np.asarray(bg, dtype=np.float32)
    We = np.asarray(We, dtype=np.float32)
    be = np.asarray(be, dtype=np.float32)
    # Pre-rearranged gating weights: partition-major [128, NIT*E] so the
    # device load is a single contiguous-per-partition DMA.
    wg_c = np.ascontiguousarray(
        Wg.reshape(NIT, 128, E).transpose(1, 0, 2).reshape(128, NIT * E))
    bg_c = np.ascontiguousarray(bg).reshape(E, 1)
    in_maps = []
    for c in range(NCORES):
        loc = list(range(EPC * c, EPC * (c + 1)))
        we_c = We[loc[0]:loc[-1] + 1].reshape(KPC, OUT)
        xs = np.ascontiguousarray(x[c * 128:(c + 1) * 128])
        in_maps.append({
            "xs": xs,
            "xts": np.ascontiguousarray(xs.T),
            "wg": wg_c,
            "bg": bg_c,
            "we": we_c.astype(BF16),
            "be": np.ascontiguousarray(be[loc[0]:loc[-1] + 1]),
        })
    return in_maps


_NC_CACHE = None


def _get_nc():
    global _NC_CACHE
    if _NC_CACHE is None:
        _NC_CACHE = build_bass()
    return _NC_CACHE


def kernel(x, Wg, bg, We, be, **_ignored):
    in_maps = make_in_maps(x, Wg, bg, We, be)
    nc = _get_nc()
    res = run_bass_kernel_spmd(nc, in_maps, core_ids=list(range(NCORES)))
    total = np.zeros(OUT, dtype=np.float32)
    for r in res.results:
        total = total + r["out"].reshape(OUT).astype(np.float32)
    return np.ascontiguousarray(
        np.broadcast_to(total, (B, OUT)).astype(np.float32)
    )
